# revision 5
# baseline (speedup 1.0000x reference)
"""Trainium2 Bass kernel for GNN copy_src -> segment-mean (dst-sharded, 8 cores).

Strategy
--------
- Partition dst nodes (and their incoming edges) across 8 NeuronCores:
  core c owns dst rows [c*6250, (c+1)*6250).
- Host-side "inspector" pass (numpy): bucket each core's edges by
  128-slot dst block, split each block's edges into two runs by src range
  (dma_gather indices are int16, so the 50000-row table is addressed as
  two halves), pad each run to a multiple of 128 with harmless dummy
  edges (src index 0, slot sentinel -1).
- Device kernel (identical SPMD program on all 8 cores):
  * dma_gather batches of source rows (512 B each) from the full
    author_emb table in HBM into SBUF.
  * per 128-edge group, build a one-hot matrix H[edge, slot] on the DVE
    via is_equal(slot_value, iota_row); dummy edges give all-zero rows.
  * TensorE matmuls accumulate H^T @ G (feature sums) and H^T @ 1
    (degree) in PSUM per 128-slot block.
  * per block: clamp degree to >=1, divide, DMA the [128, 128] result
    tile to the output shard.
- Host gathers the 8 output shards into the full [50000, 128] output.
"""

import os
import sys

import numpy as np

for _p in ("/opt/trn_rl_repo",):
    if os.path.isdir(_p) and _p not in sys.path:
        sys.path.insert(0, _p)

from concourse import bacc, mybir  # noqa: E402
import concourse.bass as bass  # noqa: E402
import concourse.tile as tile  # noqa: E402
from concourse.bass_utils import run_bass_kernel_spmd  # noqa: E402

N_NODES = 50000
N_EDGES = 600000
D_FEAT = 128
N_CORES = 8
NLOC = N_NODES // N_CORES          # 6250 dst nodes per core
BLK = 128                          # dst slots per PSUM block
NB = (NLOC + BLK - 1) // BLK       # 49 blocks per core
HALF = 32768                       # int16 index limit for dma_gather
CALLG = 8                          # groups (of 128 rows) per dma_gather call (>8 overflows the SWDGE ring ucode on HW)
SWDGE_SCRATCH = 16384              # SWDGE descriptor ring: bytes/partition (desc capacity = /16)

_cache = {}


def _prepare(src, dst):
    """Inspector pass: group/pad edges per (core, block, src-half).

    Returns per-core device arrays plus the (core-invariant) group layout.
    """
    core = dst // NLOC
    slot = dst % NLOC
    blk = slot // BLK
    srel = (slot % BLK).astype(np.float32)
    half = (src >= HALF).astype(np.int64)

    cnt = np.zeros((N_CORES, NB, 2), dtype=np.int64)
    np.add.at(cnt, (core, blk, half), 1)
    # groups per (block, half): shared across cores so the SPMD program is identical
    g = (cnt + 127) // 128
    g = g.max(axis=0)  # [NB, 2]
    # every block needs at least one matmul to initialize its PSUM tile
    zero_blocks = g.sum(axis=1) == 0
    g[zero_blocks, 0] = 1

    aoff = np.concatenate([[0], np.cumsum(g[:, 0])])  # A-list group offsets per block
    boff = np.concatenate([[0], np.cumsum(g[:, 1])])
    GA, GB = int(aoff[-1]), int(boff[-1])
    G = GA + GB

    # sort edges by (core, blk, half) once; then slice segments
    key = ((core * NB + blk) * 2 + half)
    order = np.argsort(key, kind="stable")
    key_sorted = key[order]
    src_sorted = src[order]
    srel_sorted = srel[order]
    seg_starts = np.searchsorted(key_sorted, np.arange(N_CORES * NB * 2))
    seg_ends = np.searchsorted(key_sorted, np.arange(N_CORES * NB * 2), side="right")

    idx_vals = np.zeros((N_CORES, G, 128), dtype=np.int16)
    slot_vals = np.full((N_CORES, G, 128), -1.0, dtype=np.float32)
    for c in range(N_CORES):
        for b in range(NB):
            for h in range(2):
                s, e = seg_starts[(c * NB + b) * 2 + h], seg_ends[(c * NB + b) * 2 + h]
                n = e - s
                if n == 0:
                    continue
                ng = int(g[b, h])
                g0 = (aoff[b] if h == 0 else GA + boff[b])
                iv = idx_vals[c, g0:g0 + ng].reshape(-1)
                sv = slot_vals[c, g0:g0 + ng].reshape(-1)
                sseg = src_sorted[s:e]
                iv[:n] = (sseg - HALF * h).astype(np.int16)
                sv[:n] = srel_sorted[s:e]

    # wrapped int16 layout for dma_gather: value (g, q) -> [q%16, 8*g + q//16],
    # replicated across the 8 sixteen-partition stripes
    w = idx_vals.reshape(N_CORES, G, 8, 16).transpose(0, 3, 1, 2).reshape(N_CORES, 16, G * 8)
    idxw = np.tile(w, (1, 8, 1))                       # [C, 128, G*8] int16
    slotw = slot_vals.transpose(0, 2, 1).copy()        # [C, 128, G] f32

    layout = dict(g=g, aoff=aoff, boff=boff, GA=GA, GB=GB)
    return idxw, slotw, layout


def _build_program(layout):
    g, aoff, boff = layout["g"], layout["aoff"], layout["boff"]
    GA, GB = layout["GA"], layout["GB"]
    G = GA + GB
    f32 = mybir.dt.float32

    nc = bacc.Bacc("TRN2", target_bir_lowering=False, debug=False,
                   num_devices=N_CORES, dynamic_dma_scratch_size=SWDGE_SCRATCH)
    # two separate tensors: dma_gather's ucode mishandles nonzero source-AP
    # offsets on HW, so each int16-addressable half gets its own tensor
    embA = nc.dram_tensor("embA", [HALF, D_FEAT], f32, kind="ExternalInput").ap()
    embB = nc.dram_tensor("embB", [N_NODES - HALF, D_FEAT], f32, kind="ExternalInput").ap()
    iota = nc.dram_tensor("iota", [128, BLK], f32, kind="ExternalInput").ap()
    idxw = nc.dram_tensor("idxw", [128, G * 8], mybir.dt.int16, kind="ExternalInput").ap()
    slotw = nc.dram_tensor("slotw", [128, G], f32, kind="ExternalInput").ap()
    out = nc.dram_tensor("out", [NLOC, D_FEAT], f32, kind="ExternalOutput").ap()

    # per-list gather call ranges: (list, group0, ngroups)
    calls = {0: [], 1: []}
    for lst, total in ((0, GA), (1, GB)):
        g0 = 0
        while g0 < total:
            ncg = min(CALLG, total - g0)
            calls[lst].append((g0, ncg))
            g0 += ncg

    with tile.TileContext(nc) as tc:
        with (
            tc.tile_pool(name="const", bufs=1) as cpool,
            tc.tile_pool(name="gath", bufs=3) as gpool,
            tc.tile_pool(name="hbuf", bufs=4) as hpool,
            tc.tile_pool(name="evict", bufs=3) as epool,
            tc.tile_pool(name="psum", bufs=2, space="PSUM") as ppool,
        ):
            iota_sb = cpool.tile([128, BLK], f32, tag="iota")
            nc.sync.dma_start(out=iota_sb[:], in_=iota[:])
            ones_sb = cpool.tile([128, 1], f32, tag="ones")
            nc.vector.memset(ones_sb[:], 1.0)
            idx_sb = cpool.tile([128, G * 8], mybir.dt.int16, tag="idx")
            nc.sync.dma_start(out=idx_sb[:], in_=idxw[:])
            slot_sb = cpool.tile([128, G], f32, tag="slot")
            nc.sync.dma_start(out=slot_sb[:], in_=slotw[:])

            # gather-call state per list: (call_index_issued_up_to, tile)
            cur = {0: [-1, None], 1: [-1, None]}
            srcs = {0: embA, 1: embB}

            def get_group_rhs(lst, gg):
                """Ensure the gather call containing group gg of list lst is
                issued; return the [128, 128] rhs AP for that group."""
                ci = gg // CALLG
                if cur[lst][0] != ci:
                    g0, ncg = calls[lst][ci]
                    t = gpool.tile([128, CALLG * 128], f32, tag=f"g{lst}")
                    col0 = (g0 if lst == 0 else GA + g0) * 8
                    nc.gpsimd.dma_gather(
                        out_ap=t[:, :ncg * 128].rearrange("p (n e) -> p n e", e=128),
                        in_ap=srcs[lst],
                        idxs_ap=idx_sb[:, col0:col0 + ncg * 8],
                        num_idxs=ncg * 128,
                        num_idxs_reg=ncg * 128,
                        elem_size=D_FEAT,
                    )
                    cur[lst] = [ci, t]
                loc = gg % CALLG
                return cur[lst][1][:, loc * 128:(loc + 1) * 128]

            for b in range(NB):
                groups = [(0, int(aoff[b]) + k) for k in range(int(g[b, 0]))]
                groups += [(1, int(boff[b]) + k) for k in range(int(g[b, 1]))]
                psum_s = ppool.tile([128, BLK], f32, tag="ps")
                psum_d = ppool.tile([128, 1], f32, tag="pd")
                last = len(groups) - 1
                for k, (lst, gg) in enumerate(groups):
                    rhs = get_group_rhs(lst, gg)
                    scol = gg if lst == 0 else GA + gg
                    h = hpool.tile([128, BLK], f32, tag="h")
                    nc.vector.tensor_tensor(
                        out=h[:],
                        in0=slot_sb[:, scol:scol + 1].to_broadcast([128, BLK]),
                        in1=iota_sb[:],
                        op=mybir.AluOpType.is_equal,
                    )
                    nc.tensor.matmul(out=psum_s[:], lhsT=h[:], rhs=rhs,
                                     start=(k == 0), stop=(k == last))
                    nc.tensor.matmul(out=psum_d[:], lhsT=h[:], rhs=ones_sb[:],
                                     start=(k == 0), stop=(k == last))
                degc = epool.tile([128, 1], f32, tag="deg")
                nc.vector.tensor_scalar_max(degc[:], psum_d[:], 1.0)
                # r = 1/deg with one Newton step (deg is a small integer, so
                # this is exact to fp32 even if InstReciprocal is approximate)
                r0 = epool.tile([128, 1], f32, tag="r0")
                nc.vector.reciprocal(r0[:], degc[:])
                dr = epool.tile([128, 1], f32, tag="dr")
                nc.vector.tensor_tensor(out=dr[:], in0=degc[:], in1=r0[:],
                                        op=mybir.AluOpType.mult)
                a = epool.tile([128, 1], f32, tag="nta")
                nc.vector.tensor_tensor(out=a[:], in0=dr[:], in1=r0[:],
                                        op=mybir.AluOpType.mult)
                b2 = epool.tile([128, 1], f32, tag="ntb")
                nc.vector.tensor_scalar(out=b2[:], in0=r0[:], scalar1=2.0,
                                        scalar2=None, op0=mybir.AluOpType.mult)
                r1 = epool.tile([128, 1], f32, tag="r1")
                nc.vector.tensor_tensor(out=r1[:], in0=b2[:], in1=a[:],
                                        op=mybir.AluOpType.subtract)
                ot = epool.tile([128, BLK], f32, tag="ot")
                nc.vector.tensor_scalar(
                    out=ot[:], in0=psum_s[:], scalar1=r1[:],
                    scalar2=None, op0=mybir.AluOpType.mult,
                )
                rows = min(BLK, NLOC - b * BLK)
                nc.sync.dma_start(out=out[b * BLK:b * BLK + rows, :],
                                  in_=ot[:rows, :])

    nc.compile()
    return nc


def kernel(author_emb, src, dst, n_nodes):
    emb = np.ascontiguousarray(np.asarray(author_emb, dtype=np.float32))
    src = np.asarray(src).astype(np.int64)
    dst = np.asarray(dst).astype(np.int64)
    assert emb.shape == (N_NODES, D_FEAT) and src.shape == (N_EDGES,)

    idxw, slotw, layout = _prepare(src, dst)
    key = (layout["GA"], layout["GB"], layout["g"].tobytes())
    if key not in _cache:
        _cache[key] = _build_program(layout)
    nc = _cache[key]

    iota_np = np.broadcast_to(np.arange(BLK, dtype=np.float32), (128, BLK)).copy()
    embA = np.ascontiguousarray(emb[:HALF])
    embB = np.ascontiguousarray(emb[HALF:])
    in_maps = [
        {"embA": embA, "embB": embB, "iota": iota_np, "idxw": idxw[c], "slotw": slotw[c]}
        for c in range(N_CORES)
    ]
    res = run_bass_kernel_spmd(nc, in_maps, list(range(N_CORES)))
    out = np.empty((N_NODES, D_FEAT), dtype=np.float32)
    for c in range(N_CORES):
        out[c * NLOC:(c + 1) * NLOC] = res.results[c]["out"]
    return out


# revision 7
# speedup vs baseline: 6.1222x; 6.1222x over previous
"""Trainium2 Bass kernel for GNN copy_src -> segment-mean (dst-sharded, 8 cores).

Strategy
--------
- Partition dst nodes (and their incoming edges) across 8 NeuronCores:
  core c owns dst rows [c*6250, (c+1)*6250).
- Host-side "inspector" pass (numpy): bucket each core's edges by
  128-slot dst block, split each block's edges into two runs by src range
  (dma_gather indices are int16, so the 50000-row table is addressed as
  two halves), pad each run to a multiple of 128 with harmless dummy
  edges (src index 0, slot sentinel -1).
- Device kernel (identical SPMD program on all 8 cores):
  * dma_gather batches of source rows (512 B each) from the full
    author_emb table in HBM into SBUF.
  * per 128-edge group, build a one-hot matrix H[edge, slot] on the DVE
    via is_equal(slot_value, iota_row); dummy edges give all-zero rows.
  * TensorE matmuls accumulate H^T @ G (feature sums) and H^T @ 1
    (degree) in PSUM per 128-slot block.
  * per block: clamp degree to >=1, divide, DMA the [128, 128] result
    tile to the output shard.
- Host gathers the 8 output shards into the full [50000, 128] output.
"""

import os
import sys

import numpy as np

for _p in ("/opt/trn_rl_repo",):
    if os.path.isdir(_p) and _p not in sys.path:
        sys.path.insert(0, _p)

from concourse import bacc, mybir  # noqa: E402
import concourse.bass as bass  # noqa: E402
import concourse.tile as tile  # noqa: E402
from concourse.bass_utils import run_bass_kernel_spmd  # noqa: E402

N_NODES = 50000
N_EDGES = 600000
D_FEAT = 128
N_CORES = 8
NLOC = N_NODES // N_CORES          # 6250 dst nodes per core
BLK = 128                          # dst slots per PSUM block
NB = (NLOC + BLK - 1) // BLK       # 49 blocks per core
HALF = 32768                       # int16 index limit for dma_gather
CALLG = 8                          # groups (of 128 rows) per dma_gather call (>8 overflows the SWDGE ring ucode on HW)
SWDGE_SCRATCH = 16384              # SWDGE descriptor ring: bytes/partition (desc capacity = /16)

_cache = {}


def _prepare(src, dst):
    """Inspector pass: group/pad edges per (core, block, src-half).

    Returns per-core device arrays plus the (core-invariant) group layout.
    """
    core = dst // NLOC
    slot = dst % NLOC
    blk = slot // BLK
    srel = (slot % BLK).astype(np.float32)
    half = (src >= HALF).astype(np.int64)

    cnt = np.zeros((N_CORES, NB, 2), dtype=np.int64)
    np.add.at(cnt, (core, blk, half), 1)
    # groups per (block, half): shared across cores so the SPMD program is identical
    g = (cnt + 127) // 128
    g = g.max(axis=0)  # [NB, 2]
    # every block needs at least one matmul to initialize its PSUM tile
    zero_blocks = g.sum(axis=1) == 0
    g[zero_blocks, 0] = 1

    aoff = np.concatenate([[0], np.cumsum(g[:, 0])])  # A-list group offsets per block
    boff = np.concatenate([[0], np.cumsum(g[:, 1])])
    GA, GB = int(aoff[-1]), int(boff[-1])
    G = GA + GB

    # sort edges by (core, blk, half) once; then slice segments
    key = ((core * NB + blk) * 2 + half)
    order = np.argsort(key, kind="stable")
    key_sorted = key[order]
    src_sorted = src[order]
    srel_sorted = srel[order]
    seg_starts = np.searchsorted(key_sorted, np.arange(N_CORES * NB * 2))
    seg_ends = np.searchsorted(key_sorted, np.arange(N_CORES * NB * 2), side="right")

    idx_vals = np.zeros((N_CORES, G, 128), dtype=np.int16)
    slot_vals = np.full((N_CORES, G, 128), -1.0, dtype=np.float32)
    for c in range(N_CORES):
        for b in range(NB):
            for h in range(2):
                s, e = seg_starts[(c * NB + b) * 2 + h], seg_ends[(c * NB + b) * 2 + h]
                n = e - s
                if n == 0:
                    continue
                ng = int(g[b, h])
                g0 = (aoff[b] if h == 0 else GA + boff[b])
                iv = idx_vals[c, g0:g0 + ng].reshape(-1)
                sv = slot_vals[c, g0:g0 + ng].reshape(-1)
                sseg = src_sorted[s:e]
                iv[:n] = (sseg - HALF * h).astype(np.int16)
                sv[:n] = srel_sorted[s:e]

    # wrapped int16 layout for dma_gather: value (g, q) -> [q%16, 8*g + q//16],
    # replicated across the 8 sixteen-partition stripes
    w = idx_vals.reshape(N_CORES, G, 8, 16).transpose(0, 3, 1, 2).reshape(N_CORES, 16, G * 8)
    idxw = np.tile(w, (1, 8, 1))                       # [C, 128, G*8] int16
    slotw = slot_vals.transpose(0, 2, 1).copy()        # [C, 128, G] f32

    layout = dict(g=g, aoff=aoff, boff=boff, GA=GA, GB=GB)
    return idxw, slotw, layout


def _build_program(layout):
    g, aoff, boff = layout["g"], layout["aoff"], layout["boff"]
    GA, GB = layout["GA"], layout["GB"]
    G = GA + GB
    f32 = mybir.dt.float32

    nc = bacc.Bacc("TRN2", target_bir_lowering=False, debug=False,
                   num_devices=N_CORES, dynamic_dma_scratch_size=SWDGE_SCRATCH)
    # two separate tensors: dma_gather's ucode mishandles nonzero source-AP
    # offsets on HW, so each int16-addressable half gets its own tensor
    embA = nc.dram_tensor("embA", [HALF, D_FEAT], f32, kind="ExternalInput").ap()
    embB = nc.dram_tensor("embB", [N_NODES - HALF, D_FEAT], f32, kind="ExternalInput").ap()
    iota = nc.dram_tensor("iota", [128, BLK], f32, kind="ExternalInput").ap()
    idxw = nc.dram_tensor("idxw", [128, G * 8], mybir.dt.int16, kind="ExternalInput").ap()
    slotw = nc.dram_tensor("slotw", [128, G], f32, kind="ExternalInput").ap()
    out = nc.dram_tensor("out", [NLOC, D_FEAT], f32, kind="ExternalOutput").ap()

    # per-list gather call ranges: (list, group0, ngroups)
    calls = {0: [], 1: []}
    for lst, total in ((0, GA), (1, GB)):
        g0 = 0
        while g0 < total:
            ncg = min(CALLG, total - g0)
            calls[lst].append((g0, ncg))
            g0 += ncg

    with tile.TileContext(nc) as tc:
        with (
            tc.tile_pool(name="const", bufs=1) as cpool,
            tc.tile_pool(name="gath", bufs=4) as gpool,
            tc.tile_pool(name="hbuf", bufs=6) as hpool,
            tc.tile_pool(name="evict", bufs=3) as epool,
            tc.tile_pool(name="psum", bufs=4, space="PSUM") as ppool,
        ):
            iota_sb = cpool.tile([128, BLK], f32, tag="iota")
            nc.sync.dma_start(out=iota_sb[:], in_=iota[:])
            ones_sb = cpool.tile([128, 1], f32, tag="ones")
            nc.vector.memset(ones_sb[:], 1.0)
            idx_sb = cpool.tile([128, G * 8], mybir.dt.int16, tag="idx")
            nc.sync.dma_start(out=idx_sb[:], in_=idxw[:])
            slot_sb = cpool.tile([128, G], f32, tag="slot")
            nc.sync.dma_start(out=slot_sb[:], in_=slotw[:])

            # gather-call state per list: (call_index_issued_up_to, tile)
            cur = {0: [-1, None], 1: [-1, None]}
            srcs = {0: embA, 1: embB}

            def get_group_rhs(lst, gg):
                """Ensure the gather call containing group gg of list lst is
                issued; return the [128, 128] rhs AP for that group."""
                ci = gg // CALLG
                if cur[lst][0] != ci:
                    g0, ncg = calls[lst][ci]
                    t = gpool.tile([128, CALLG * 128], f32, tag=f"g{lst}")
                    col0 = (g0 if lst == 0 else GA + g0) * 8
                    nc.gpsimd.dma_gather(
                        out_ap=t[:, :ncg * 128].rearrange("p (n e) -> p n e", e=128),
                        in_ap=srcs[lst],
                        idxs_ap=idx_sb[:, col0:col0 + ncg * 8],
                        num_idxs=ncg * 128,
                        num_idxs_reg=ncg * 128,
                        elem_size=D_FEAT,
                    )
                    cur[lst] = [ci, t]
                loc = gg % CALLG
                return cur[lst][1][:, loc * 128:(loc + 1) * 128]

            for b in range(NB):
                groups = [(0, int(aoff[b]) + k) for k in range(int(g[b, 0]))]
                groups += [(1, int(boff[b]) + k) for k in range(int(g[b, 1]))]
                psum_s = ppool.tile([128, BLK], f32, tag="ps")
                psum_d = ppool.tile([128, 1], f32, tag="pd")
                last = len(groups) - 1
                for k, (lst, gg) in enumerate(groups):
                    rhs = get_group_rhs(lst, gg)
                    scol = gg if lst == 0 else GA + gg
                    h = hpool.tile([128, BLK], f32, tag="h")
                    nc.vector.tensor_tensor(
                        out=h[:],
                        in0=slot_sb[:, scol:scol + 1].to_broadcast([128, BLK]),
                        in1=iota_sb[:],
                        op=mybir.AluOpType.is_equal,
                    )
                    nc.tensor.matmul(out=psum_s[:], lhsT=h[:], rhs=rhs,
                                     start=(k == 0), stop=(k == last))
                    nc.tensor.matmul(out=psum_d[:], lhsT=h[:], rhs=ones_sb[:],
                                     start=(k == 0), stop=(k == last))
                degc = epool.tile([128, 1], f32, tag="deg")
                nc.vector.tensor_scalar_max(degc[:], psum_d[:], 1.0)
                # r = 1/deg with one Newton step (deg is a small integer, so
                # this is exact to fp32 even if InstReciprocal is approximate)
                r0 = epool.tile([128, 1], f32, tag="r0")
                nc.vector.reciprocal(r0[:], degc[:])
                dr = epool.tile([128, 1], f32, tag="dr")
                nc.vector.tensor_tensor(out=dr[:], in0=degc[:], in1=r0[:],
                                        op=mybir.AluOpType.mult)
                a = epool.tile([128, 1], f32, tag="nta")
                nc.vector.tensor_tensor(out=a[:], in0=dr[:], in1=r0[:],
                                        op=mybir.AluOpType.mult)
                b2 = epool.tile([128, 1], f32, tag="ntb")
                nc.vector.tensor_scalar(out=b2[:], in0=r0[:], scalar1=2.0,
                                        scalar2=None, op0=mybir.AluOpType.mult)
                r1 = epool.tile([128, 1], f32, tag="r1")
                nc.vector.tensor_tensor(out=r1[:], in0=b2[:], in1=a[:],
                                        op=mybir.AluOpType.subtract)
                ot = epool.tile([128, BLK], f32, tag="ot")
                nc.vector.tensor_scalar(
                    out=ot[:], in0=psum_s[:], scalar1=r1[:],
                    scalar2=None, op0=mybir.AluOpType.mult,
                )
                rows = min(BLK, NLOC - b * BLK)
                nc.sync.dma_start(out=out[b * BLK:b * BLK + rows, :],
                                  in_=ot[:rows, :])

    nc.compile()
    return nc


def kernel(author_emb, src, dst, n_nodes):
    emb = np.ascontiguousarray(np.asarray(author_emb, dtype=np.float32))
    src = np.asarray(src).astype(np.int64)
    dst = np.asarray(dst).astype(np.int64)
    assert emb.shape == (N_NODES, D_FEAT) and src.shape == (N_EDGES,)

    idxw, slotw, layout = _prepare(src, dst)
    key = (layout["GA"], layout["GB"], layout["g"].tobytes())
    if key not in _cache:
        _cache[key] = _build_program(layout)
    nc = _cache[key]

    iota_np = np.broadcast_to(np.arange(BLK, dtype=np.float32), (128, BLK)).copy()
    embA = np.ascontiguousarray(emb[:HALF])
    embB = np.ascontiguousarray(emb[HALF:])
    in_maps = [
        {"embA": embA, "embB": embB, "iota": iota_np, "idxw": idxw[c], "slotw": slotw[c]}
        for c in range(N_CORES)
    ]
    res = run_bass_kernel_spmd(nc, in_maps, list(range(N_CORES)))
    out = np.empty((N_NODES, D_FEAT), dtype=np.float32)
    for c in range(N_CORES):
        out[c * NLOC:(c + 1) * NLOC] = res.results[c]["out"]
    return out
